# revision 31
# baseline (speedup 1.0000x reference)
"""Differential multi-head self-attention on 8 Trainium2 NeuronCores.

Sharding: core c handles batch b = c // 4 and heads {2*(c%4), 2*(c%4)+1}
(data parallel over batch, tensor parallel over heads). One SPMD Bass
program runs on all 8 cores; per-core differences flow through input data.

v2 design (all-bf16 matmuls, softmax halves combined before AV):
  host supplies xT = x[b].T in bf16 (pure layout prep - no transposes on PE)
  KT = Wk.T @ xT, QT per tile, V = x @ Wv     (bf16 matmuls, FWL weight loads)
  per q tile of 256 q rows, k-chunk-major:
    scores[k, 2*256q] for both halves -> one exp per chunk (ACT)
    sums[1, 512] += ones.T @ E        (rank-1-stationary matmul per chunk)
  g[q] = lam * s1[q] / s2[q]; G = broadcast(g) via rank-1 matmul
  W = E1 - G.*E2  (Pool multiply + DVE subtract, bf16)
    -- exploits RMS-norm scale invariance: O = W@V = s1 * (a1 - lam*a2)@V,
       and the per-row factor s1 cancels in O/rms(O).
  OT[d, q] += V_chunk.T @ W            (transpose-free AV orientation)
  rms: sq = (OT/sqrt(D))^2 (ACT), ms = ones.T @ sq (matmul), rr = rsqrt
  out.T[e, q] = (Wo.T @ OT) * rr       (per-q rr applied on the PSUM drain)
  DRAM out is [H, E, S]; the host transposes when gathering (free).
"""

import numpy as np
import ml_dtypes

import concourse.bass as bass
import concourse.mybir as mybir
import concourse.tile as tile
from concourse import bacc
from concourse.bass_utils import run_bass_kernel_spmd
from concourse.hw_specs import get_activation_tables

B, S, E, H, D = 2, 2048, 512, 8, 512
HALF = D // 2
HLOC = 2            # heads per core
NCORES = 8
QT = 256            # q tile
NQT = S // QT       # 8
KC = 128            # k chunk
NKC = S // KC       # 16
NDC = D // 128      # 4
NEC = E // 128      # 4
NSC = S // 128      # 16
SCALE = 1.0 / float(np.sqrt(HALF))
EPS = float(np.finfo(np.float32).eps)
LAMBDA_INIT = 0.8

f32 = mybir.dt.float32
bf16 = mybir.dt.bfloat16
AF = mybir.ActivationFunctionType
ALU = mybir.AluOpType

SKIP, FULL = -1, -2


def _analyze_mask(mask):
    """Per (q-tile, k-chunk) block status: SKIP / FULL / keep-pattern index."""
    status = [[SKIP] * NKC for _ in range(NQT)]
    pats = []
    pat_idx = {}
    for t in range(NQT):
        for kc in range(NKC):
            blk = mask[t * QT:(t + 1) * QT, kc * KC:(kc + 1) * KC]  # [256 q, 128 k]
            if blk.all():
                status[t][kc] = SKIP
            elif not blk.any():
                status[t][kc] = FULL
            else:
                keep = (~blk).T.astype(np.float32)  # [128 k, 256 q]
                key = keep.tobytes()
                if key not in pat_idx:
                    pat_idx[key] = len(pats)
                    pats.append(keep)
                status[t][kc] = pat_idx[key]
    return status, pats


def _build(status, npat, hv_pat=(), repeat=1, unroll=1):  # noqa: C901
    nc = bacc.Bacc("TRN2", target_bir_lowering=False, debug=False)

    xt_d = nc.dram_tensor("xt", [E, S], bf16, kind="ExternalInput")
    wq_d = nc.dram_tensor("wq", [HLOC, E, D], bf16, kind="ExternalInput")
    wk_d = nc.dram_tensor("wk", [HLOC, E, D], bf16, kind="ExternalInput")
    wv_d = nc.dram_tensor("wv", [HLOC, E, D], bf16, kind="ExternalInput")
    wo_d = nc.dram_tensor("wo", [HLOC, D, E], bf16, kind="ExternalInput")
    lam_d = nc.dram_tensor("lam", [HLOC, 1, 1], f32, kind="ExternalInput")
    keeps_d = nc.dram_tensor("keeps", [npat, 128, QT], bf16, kind="ExternalInput")
    out_d = nc.dram_tensor("out", [HLOC, E, S], f32, kind="ExternalOutput")
    iters_d = nc.dram_tensor("iters", [1, 1], f32, kind="ExternalOutput") if repeat > 1 else None

    act_sets = list(get_activation_tables(nc.m.arch).keys())
    nle_set = act_sets.index("natural_log_exp_and_others")

    with tile.TileContext(nc) as tc:
        with tc.tile_pool(name="cst", bufs=1) as cst, \
             tc.tile_pool(name="xtp", bufs=1) as xtp, \
             tc.tile_pool(name="ktp", bufs=2) as ktp, \
             tc.tile_pool(name="vtp", bufs=2) as vtp, \
             tc.tile_pool(name="wts", bufs=2) as wts, \
             tc.tile_pool(name="qtp", bufs=1) as qtp, \
             tc.tile_pool(name="epool", bufs=2) as epool, \
             tc.tile_pool(name="wpool", bufs=3) as wpool, \
             tc.tile_pool(name="scr", bufs=2) as scr, \
             tc.tile_pool(name="sg", bufs=2, space="PSUM") as sg, \
             tc.tile_pool(name="pot", bufs=1, space="PSUM") as pot, \
             tc.tile_pool(name="pss", bufs=1, space="PSUM") as pss, \
             tc.tile_pool(name="ps", bufs=3, space="PSUM") as ps:

            # One ACT table set covers Exp/Ln/Square/Copy: load it up front.
            nc.scalar.add_instruction(mybir.InstLoadActFuncSet(
                name=nc.get_next_instruction_name(),
                ins=[], outs=[], act_func_set_id=nle_set))

            ones_col = cst.tile([128, 1], bf16, tag="ones_col")
            nc.gpsimd.memset(ones_col[:], 1.0)
            ones_row = cst.tile([1, 128], bf16, tag="ones_row")
            nc.gpsimd.memset(ones_row[:], 1.0)
            keeps_t = cst.tile([128, max(npat, 1), QT], bf16, tag="keeps")
            for i in range(npat):
                nc.sync.dma_start(keeps_t[:, i, :], keeps_d.ap()[i])
            lam_t = cst.tile([1, HLOC], f32, tag="lam")
            for h in range(HLOC):
                nc.sync.dma_start(lam_t[:, h:h + 1], lam_d.ap()[h])
            eps_t = cst.tile([1, 1], f32, tag="eps")
            nc.gpsimd.memset(eps_t[:], EPS)

            if repeat > 1:
                ctr = cst.tile([1, 1], f32, tag="ctr")
                nc.gpsimd.memset(ctr[:], 0.0)
            rep_ctx = tc.For_i(0, repeat, 1) if repeat > 1 else None
            if rep_ctx is not None:
                rep_ctx.__enter__()
                nc.vector.tensor_scalar_add(ctr[:], ctr[:], 1.0)

            def emit_body():
                # input DMAs issue from the Pool queue so they don't wait
                # behind the previous iteration's output DMAs on SP; wk/wq of
                # head 0 lead so the first KT matmuls start ASAP
                xT = xtp.tile([128, NEC, S], bf16, tag="xT")
                wtiles = []
                for h in range(HLOC):
                    wq_t = wts.tile([128, NEC, D], bf16, tag="wq", name=f"wq{h}")
                    wk_t = wts.tile([128, NEC, D], bf16, tag="wk", name=f"wk{h}")
                    wv_t = wts.tile([128, NEC, D], bf16, tag="wv", name=f"wv{h}")
                    wo_t = wts.tile([128, NDC, E], bf16, tag="wo", name=f"wo{h}")
                    wtiles.append((wq_t, wk_t, wv_t, wo_t))
                for ec in range(NEC):
                    nc.gpsimd.dma_start(wtiles[0][1][:, ec, :], wk_d.ap()[0, ec * 128:(ec + 1) * 128, :])
                for ec in range(NEC):
                    nc.gpsimd.dma_start(xT[:, ec, :], xt_d.ap()[ec * 128:(ec + 1) * 128, :])
                for h in range(HLOC):
                    wq_t, wk_t, wv_t, wo_t = wtiles[h]
                    for ec in range(NEC):
                        if h > 0:
                            nc.gpsimd.dma_start(wk_t[:, ec, :], wk_d.ap()[h, ec * 128:(ec + 1) * 128, :])
                        nc.gpsimd.dma_start(wq_t[:, ec, :], wq_d.ap()[h, ec * 128:(ec + 1) * 128, :])
                        nc.gpsimd.dma_start(wv_t[:, ec, :], wv_d.ap()[h, ec * 128:(ec + 1) * 128, :])
                    for dc in range(NDC):
                        nc.gpsimd.dma_start(wo_t[:, dc, :], wo_d.ap()[h, dc * 128:(dc + 1) * 128, :])

                # ---- software pipeline (per step t of the tile loop) ----
                #  B1(t-2): rms row math (ms matmuls + Ln/Exp)
                #  scores/exp/mask/sums(t); rec/g rows (DVE)
                #  AV(t-1) with the g-broadcast matmul for t injected after
                #    its first chain; combine(t) on Pool+DVE; sq/OT drains
                #  qt(t+1); B2(t-2): rr broadcast + outproj + store
                pend = {"A1": None, "A2": None, "B": None}

                def emit_tailA_AV(st, pre=None, inject=None):
                    h_, q0_, kcs_, qlo_, Wt_, V_, wo_t_ = st
                    nk_ = len(kcs_)
                    ot_ps = pot.tile([128, NDC, QT], f32, tag="ot")
                    for dc in range(NDC):
                        for i, kc in enumerate(kcs_):
                            lo = qlo_[kc]
                            nc.tensor.matmul(
                                ot_ps[:, dc, lo:QT],
                                V_[:, kc, dc * 128:(dc + 1) * 128],
                                Wt_[:, kc, lo:QT],
                                start=(i == 0), stop=(i == nk_ - 1))
                        if dc == 0 and pre is not None:
                            pre()
                            pre = None
                        if dc == 1 and inject is not None:
                            inject()
                            inject = None
                    if pre is not None:
                        pre()
                    if inject is not None:
                        inject()
                    return (h_, q0_, ot_ps, wo_t_)

                def emit_tailA_drains(ctx):
                    h_, q0_, ot_ps, wo_t_ = ctx
                    sq_sb = scr.tile([128, NDC, QT], bf16, tag="sq")
                    nc.scalar.activation(sq_sb[:], ot_ps[:], AF.Square,
                                         scale=float(1.0 / np.sqrt(D)))
                    ot_sb = scr.tile([128, NDC, QT], bf16, tag="otsb")
                    nc.vector.tensor_copy(ot_sb[:], ot_ps[:])
                    return (h_, q0_, sq_sb, ot_sb, wo_t_)

                def emit_tailB1(st):
                    h_, q0_, sq_sb_, ot_sb_, wo_t_ = st
                    ms_ps = pss.tile([1, QT], f32, tag="sums", name="ms_ps")
                    for dc in range(NDC):
                        nc.tensor.matmul(ms_ps[:], ones_col[:], sq_sb_[:, dc, :],
                                         start=(dc == 0), stop=(dc == NDC - 1))
                    lnm = scr.tile([1, QT], f32, tag="lnm")
                    nc.scalar.activation(lnm[:], ms_ps[:], AF.Ln, bias=eps_t[:])
                    rr = scr.tile([1, QT], bf16, tag="rr")
                    nc.scalar.activation(rr[:], lnm[:], AF.Exp, scale=-0.5)
                    return rr

                def emit_tailB2(st, rr):
                    h_, q0_, sq_sb_, ot_sb_, wo_t_ = st
                    rr_ps = ps.tile([128, QT], f32, tag="mmps", name="rrps")
                    nc.tensor.matmul(rr_ps[:], ones_row[:], rr[:], start=True, stop=True)
                    rr_bc = scr.tile([128, 1, QT], bf16, tag="rrbc")
                    nc.vector.tensor_copy(rr_bc[:, 0, :], rr_ps[:])
                    for ep in range(NEC // 2):
                        ops = ps.tile([128, 2, QT], f32, tag="mmps", name="outps")
                        for j in range(2):
                            ec = ep * 2 + j
                            for dc in range(NDC):
                                nc.tensor.matmul(
                                    ops[:, j, :], wo_t_[:, dc, ec * 128:(ec + 1) * 128],
                                    ot_sb_[:, dc, :],
                                    start=(dc == 0), stop=(dc == NDC - 1))
                        out_sb = scr.tile([128, 2, QT], f32, tag="outsb")
                        nc.vector.tensor_tensor(
                            out=out_sb[:], in0=ops[:],
                            in1=rr_bc[:, 0:1, :].to_broadcast([128, 2, QT]),
                            op=ALU.mult)
                        nc.sync.dma_start(
                            out_d.ap()[h_, ep * 256:(ep + 1) * 256, q0_:q0_ + QT]
                                .rearrange("(j p) q -> p j q", p=128),
                            out_sb[:])

                for h in range(HLOC):
                    wq_t, wk_t, wv_t, wo_t = wtiles[h]

                    # KT[d, k] = Wk.T @ xT   (bf16, drained on DVE)
                    KT = ktp.tile([128, NDC, S], bf16, tag="KT")
                    for dc in range(NDC):
                        for kt in range(4):
                            kps = ps.tile([128, 512], f32, tag="mmps", name="kps")
                            for ec in range(NEC):
                                nc.tensor.matmul(
                                    kps[:],
                                    wk_t[:, ec, dc * 128:(dc + 1) * 128],
                                    xT[:, ec, kt * 512:(kt + 1) * 512],
                                    start=(ec == 0), stop=(ec == NEC - 1))
                            nc.vector.tensor_copy(KT[:, dc, kt * 512:(kt + 1) * 512], kps[:])

                    # QT[d, q] = Wq.T @ xT for the whole head (bf16)
                    qt_t = qtp.tile([128, NDC, S], bf16, tag="qt")
                    for dc in range(NDC):
                        for kt in range(4):
                            qps = ps.tile([128, 512], f32, tag="mmps", name="qps")
                            for ec in range(NEC):
                                nc.tensor.matmul(
                                    qps[:],
                                    wq_t[:, ec, dc * 128:(dc + 1) * 128],
                                    xT[:, ec, kt * 512:(kt + 1) * 512],
                                    start=(ec == 0), stop=(ec == NEC - 1))
                            nc.scalar.activation(qt_t[:, dc, kt * 512:(kt + 1) * 512],
                                                 qps[:], AF.Copy)

                    # V[k, d] = x @ Wv   (bf16, drained on ACT)
                    V = vtp.tile([128, NKC, D], bf16, tag="V")
                    for sc in range(NSC):
                        vps = ps.tile([128, D], f32, tag="mmps", name="vps")
                        for ec in range(NEC):
                            nc.tensor.matmul(
                                vps[:],
                                xT[:, ec, sc * 128:(sc + 1) * 128],
                                wv_t[:, ec, :],
                                start=(ec == 0), stop=(ec == NEC - 1))
                        nc.vector.tensor_copy(V[:, sc, :], vps[:])

                    for t in range(NQT):
                        q0 = t * QT
                        diag_kcs = [kc for kc in range(NKC) if status[t][kc] >= 0]
                        full_kcs = [kc for kc in range(NKC) if status[t][kc] == FULL]
                        kcs = diag_kcs + full_kcs
                        nk = len(kcs)
                        # half-visible diagonal chunks compute only q[QT/2:)
                        # (needs a trailing full-width chunk to close groups)
                        qlo = {kc: (QT // 2 if (status[t][kc] >= 0
                                                and hv_pat[status[t][kc]]
                                                and full_kcs) else 0)
                               for kc in kcs}

                        # rms row math for tile t-2 (sq ready; before exps(t)
                        # enter the ACT queue so Ln/Exp aren't delayed)
                        rrB = None
                        if pend["B"] is not None:
                            rrB = emit_tailB1(pend["B"])

                        Et = epool.tile([128, NKC, 2, QT], bf16, tag="E")
                        Wt = wpool.tile([128, NKC, QT], bf16, tag="W")
                        sums_ps = pss.tile([1, 2, QT], f32, tag="sums")

                        # scores -> exp -> (mask); sums lag two chunks so PE
                        # never waits on the ACT exp latency
                        def emit_sums(i, kc):
                            lo = qlo[kc]
                            if lo == 0:
                                nc.tensor.matmul(
                                    sums_ps[:, :, :], ones_col[:], Et[:, kc, :, :],
                                    start=(i == 0), stop=(i == nk - 1))
                            else:
                                for half in (0, 1):
                                    nc.tensor.matmul(
                                        sums_ps[:, half, lo:QT], ones_col[:],
                                        Et[:, kc, half, lo:QT],
                                        start=(i == 0), stop=(i == nk - 1))

                        for i, kc in enumerate(kcs):
                            lo = qlo[kc]
                            sp = sg.tile([128, 2, QT], f32, tag="sp")
                            for half in (0, 1):
                                for j in (0, 1):
                                    dc = half * 2 + j
                                    nc.tensor.matmul(
                                        sp[:, half, lo:QT],
                                        KT[:, dc, kc * 128:(kc + 1) * 128],
                                        qt_t[:, dc, q0 + lo:q0 + QT],
                                        start=(j == 0), stop=(j == 1))
                            nc.scalar.activation(Et[:, kc, :, lo:QT],
                                                 sp[:, :, lo:QT], AF.Exp, scale=SCALE)
                            pat = status[t][kc]
                            if pat >= 0:
                                nc.vector.tensor_tensor(
                                    out=Et[:, kc, :, lo:QT], in0=Et[:, kc, :, lo:QT],
                                    in1=keeps_t[:, pat:pat + 1, lo:QT].to_broadcast(
                                        [128, 2, QT - lo]),
                                    op=ALU.mult)
                            if i >= 3:
                                emit_sums(i - 3, kcs[i - 3])

                        g_row = scr.tile([1, QT], bf16, tag="g")
                        g_bc = scr.tile([128, 1, QT], bf16, tag="gbc")

                        # sums tail + per-q combine scalar g = lam * s1 / s2,
                        # deferred into the AV stream so AV work covers the
                        # trailing exp latencies
                        def emit_sums_tail(emit_sums_=emit_sums, kcs_=kcs,
                                           nk_=nk, sums_ps_=sums_ps,
                                           g_row_=g_row, h_=h):
                            for i in range(max(0, nk_ - 3), nk_):
                                emit_sums_(i, kcs_[i])
                            rec = scr.tile([1, QT], f32, tag="rec")
                            nc.vector.reciprocal(rec[:], sums_ps_[:, 1, :])
                            nc.vector.scalar_tensor_tensor(
                                out=g_row_[:], in0=sums_ps_[:, 0, :],
                                scalar=lam_t[:, h_:h_ + 1], in1=rec[:],
                                op0=ALU.mult, op1=ALU.mult)

                        def emit_g_bc(g_row_=g_row, g_bc_=g_bc):
                            g_ps = ps.tile([128, QT], f32, tag="mmps", name="gps")
                            nc.tensor.matmul(g_ps[:], ones_row[:], g_row_[:],
                                             start=True, stop=True)
                            nc.vector.tensor_copy(g_bc_[:, 0, :], g_ps[:])

                        ctxA = None
                        if pend["A2"] is not None:
                            ctxA = emit_tailA_AV(pend["A2"], pre=emit_sums_tail,
                                                 inject=emit_g_bc)
                        else:
                            emit_sums_tail()
                            emit_g_bc()
                        ctxB = emit_tailA_drains(ctxA) if ctxA is not None else None

                        # W = E1 - G.*E2 over kc pairs (multiplies
                        # alternate Pool/DVE; contiguous kcs share one op)
                        pairs = []
                        ii = 0
                        while ii < nk:
                            if (ii + 1 < nk and kcs[ii + 1] == kcs[ii] + 1
                                    and qlo[kcs[ii]] == 0 and qlo[kcs[ii + 1]] == 0):
                                pairs.append((kcs[ii], 2, 0))
                                ii += 2
                            else:
                                pairs.append((kcs[ii], 1, qlo[kcs[ii]]))
                                ii += 1
                        for i, (kc, w_, lo) in enumerate(pairs):
                            tmp = scr.tile([128, 2, QT], bf16, tag="tmp")
                            eng = nc.vector if i % 2 == 0 else nc.gpsimd
                            eng.tensor_tensor(
                                out=tmp[:, 0:w_, lo:QT],
                                in0=g_bc[:, 0:1, lo:QT].to_broadcast(
                                    [128, w_, QT - lo]),
                                in1=Et[:, kc:kc + w_, 1, lo:QT], op=ALU.mult)
                            nc.vector.tensor_tensor(
                                out=Wt[:, kc:kc + w_, lo:QT],
                                in0=Et[:, kc:kc + w_, 0, lo:QT],
                                in1=tmp[:, 0:w_, lo:QT], op=ALU.subtract)
                        if pend["B"] is not None:
                            emit_tailB2(pend["B"], rrB)
                        pend["B"] = ctxB
                        pend["A2"] = pend["A1"]
                        pend["A1"] = (h, q0, kcs, qlo, Wt, V, wo_t)

                # flush: two AV stages remain (last head's tiles 6 and 7)
                for _ in range(2):
                    rrB = emit_tailB1(pend["B"]) if pend["B"] is not None else None
                    ctxA = emit_tailA_AV(pend["A2"])
                    ctxB = emit_tailA_drains(ctxA)
                    if pend["B"] is not None:
                        emit_tailB2(pend["B"], rrB)
                    pend["B"] = ctxB
                    pend["A2"] = pend["A1"]
                    pend["A1"] = None
                rrB = emit_tailB1(pend["B"])
                emit_tailB2(pend["B"], rrB)
                pend["A2"] = pend["B"] = None

            for _u in range(unroll):
                emit_body()

            if rep_ctx is not None:
                rep_ctx.__exit__(None, None, None)
                nc.sync.dma_start(iters_d.ap()[:], ctr[:])

    nc.compile()
    return nc


_CACHE = {}


def _get_program(mask, repeat=1, unroll=1):
    key = (mask.tobytes(), repeat, unroll)
    if key not in _CACHE:
        status, pats = _analyze_mask(mask)
        hv_pat = [bool(p[:, :QT // 2].max() == 0) for p in pats]
        nc = _build(status, len(pats), hv_pat=hv_pat, repeat=repeat, unroll=unroll)
        _CACHE[key] = (nc, pats)
    return _CACHE[key]


def make_in_maps(x, mask, Wq, bq, Wk, bk, Wv, bv, lq1, lk1, lq2, lk2,
                 lam_init_p, rms_w, Wo, bo, repeat=1, unroll=1):
    x = np.asarray(x, np.float32)
    mask = np.asarray(mask, bool)
    Wq = np.asarray(Wq, np.float32)
    Wk = np.asarray(Wk, np.float32)
    Wv = np.asarray(Wv, np.float32)
    Wo = np.asarray(Wo, np.float32)
    for b_ in (bq, bk, bv):
        assert np.abs(np.asarray(b_)).max() == 0.0, "nonzero qkv bias unsupported"
    lam = (np.exp((np.asarray(lq1, np.float32) * np.asarray(lk1, np.float32)).sum(-1))
           - np.exp((np.asarray(lq2, np.float32) * np.asarray(lk2, np.float32)).sum(-1))
           + np.asarray(lam_init_p, np.float32))  # [H]
    woF = Wo.reshape(H, D, E) * ((1.0 - LAMBDA_INIT) * np.asarray(rms_w, np.float32))[:, :, None]

    nc, pats = _get_program(mask, repeat=repeat, unroll=unroll)
    if pats:
        keeps = np.stack(pats).astype(ml_dtypes.bfloat16)
    else:
        keeps = np.zeros((1, 128, QT), ml_dtypes.bfloat16)

    in_maps = []
    for c in range(NCORES):
        b = c // 4
        h0 = HLOC * (c % 4)
        in_maps.append({
            "xt": np.ascontiguousarray(x[b].T).astype(ml_dtypes.bfloat16),
            "wq": Wq[h0:h0 + HLOC].astype(ml_dtypes.bfloat16),
            "wk": Wk[h0:h0 + HLOC].astype(ml_dtypes.bfloat16),
            "wv": Wv[h0:h0 + HLOC].astype(ml_dtypes.bfloat16),
            "wo": woF[h0:h0 + HLOC].astype(ml_dtypes.bfloat16),
            "lam": np.ascontiguousarray(lam[h0:h0 + HLOC].astype(np.float32)[:, None, None]),
            "keeps": keeps,
        })
    return nc, in_maps


def gather(results, bo):
    out = np.zeros((B, S, E), np.float32)
    for c in range(NCORES):
        out[c // 4] += results[c]["out"].sum(axis=0).T
    out += np.asarray(bo, np.float32)[None, None, :]
    return out


def kernel(**inputs):
    nc, in_maps = make_in_maps(**inputs)
    res = run_bass_kernel_spmd(nc, in_maps, core_ids=list(range(NCORES)))
    return gather(res.results, inputs["bo"])


# revision 32
# speedup vs baseline: 1.1101x; 1.1101x over previous
"""Differential multi-head self-attention on 8 Trainium2 NeuronCores.

Sharding: core c handles batch b = c // 4 and heads {2*(c%4), 2*(c%4)+1}
(data parallel over batch, tensor parallel over heads). One SPMD Bass
program runs on all 8 cores; per-core differences flow through input data.

v2 design (all-bf16 matmuls, softmax halves combined before AV):
  host supplies xT = x[b].T in bf16 (pure layout prep - no transposes on PE)
  KT = Wk.T @ xT, QT per tile, V = x @ Wv     (bf16 matmuls, FWL weight loads)
  per q tile of 256 q rows, k-chunk-major:
    scores[k, 2*256q] for both halves -> one exp per chunk (ACT)
    sums[1, 512] += ones.T @ E        (rank-1-stationary matmul per chunk)
  g[q] = lam * s1[q] / s2[q]; G = broadcast(g) via rank-1 matmul
  W = E1 - G.*E2  (Pool multiply + DVE subtract, bf16)
    -- exploits RMS-norm scale invariance: O = W@V = s1 * (a1 - lam*a2)@V,
       and the per-row factor s1 cancels in O/rms(O).
  OT[d, q] += V_chunk.T @ W            (transpose-free AV orientation)
  rms: sq = (OT/sqrt(D))^2 (ACT), ms = ones.T @ sq (matmul), rr = rsqrt
  out.T[e, q] = (Wo.T @ OT) * rr       (per-q rr applied on the PSUM drain)
  DRAM out is [H, E, S]; the host transposes when gathering (free).
"""

import numpy as np
import ml_dtypes

import concourse.bass as bass
import concourse.mybir as mybir
import concourse.tile as tile
from concourse import bacc
from concourse.bass_utils import run_bass_kernel_spmd
from concourse.hw_specs import get_activation_tables

B, S, E, H, D = 2, 2048, 512, 8, 512
HALF = D // 2
HLOC = 2            # heads per core
NCORES = 8
QT = 256            # q tile
NQT = S // QT       # 8
KC = 128            # k chunk
NKC = S // KC       # 16
NDC = D // 128      # 4
NEC = E // 128      # 4
NSC = S // 128      # 16
SCALE = 1.0 / float(np.sqrt(HALF))
EPS = float(np.finfo(np.float32).eps)
LAMBDA_INIT = 0.8

f32 = mybir.dt.float32
bf16 = mybir.dt.bfloat16
AF = mybir.ActivationFunctionType
ALU = mybir.AluOpType

SKIP, FULL = -1, -2


def _analyze_mask(mask):
    """Per (q-tile, k-chunk) block status: SKIP / FULL / keep-pattern index."""
    status = [[SKIP] * NKC for _ in range(NQT)]
    pats = []
    pat_idx = {}
    for t in range(NQT):
        for kc in range(NKC):
            blk = mask[t * QT:(t + 1) * QT, kc * KC:(kc + 1) * KC]  # [256 q, 128 k]
            if blk.all():
                status[t][kc] = SKIP
            elif not blk.any():
                status[t][kc] = FULL
            else:
                keep = (~blk).T.astype(np.float32)  # [128 k, 256 q]
                key = keep.tobytes()
                if key not in pat_idx:
                    pat_idx[key] = len(pats)
                    pats.append(keep)
                status[t][kc] = pat_idx[key]
    return status, pats


def _build(status, npat, hv_pat=(), repeat=1, unroll=1):  # noqa: C901
    nc = bacc.Bacc("TRN2", target_bir_lowering=False, debug=False)

    xt_d = nc.dram_tensor("xt", [E, S], bf16, kind="ExternalInput")
    wq_d = nc.dram_tensor("wq", [HLOC, E, D], bf16, kind="ExternalInput")
    wk_d = nc.dram_tensor("wk", [HLOC, E, D], bf16, kind="ExternalInput")
    wv_d = nc.dram_tensor("wv", [HLOC, E, D], bf16, kind="ExternalInput")
    wo_d = nc.dram_tensor("wo", [HLOC, D, E], bf16, kind="ExternalInput")
    lam_d = nc.dram_tensor("lam", [HLOC, 1, 1], f32, kind="ExternalInput")
    keeps_d = nc.dram_tensor("keeps", [npat, 128, QT], bf16, kind="ExternalInput")
    out_d = nc.dram_tensor("out", [HLOC, E, S], f32, kind="ExternalOutput")
    iters_d = nc.dram_tensor("iters", [1, 1], f32, kind="ExternalOutput") if repeat > 1 else None

    act_sets = list(get_activation_tables(nc.m.arch).keys())
    nle_set = act_sets.index("natural_log_exp_and_others")

    with tile.TileContext(nc) as tc:
        with tc.tile_pool(name="cst", bufs=1) as cst, \
             tc.tile_pool(name="xtp", bufs=1) as xtp, \
             tc.tile_pool(name="ktp", bufs=2) as ktp, \
             tc.tile_pool(name="vtp", bufs=2) as vtp, \
             tc.tile_pool(name="wts", bufs=2) as wts, \
             tc.tile_pool(name="qtp", bufs=1) as qtp, \
             tc.tile_pool(name="epool", bufs=2) as epool, \
             tc.tile_pool(name="wpool", bufs=3) as wpool, \
             tc.tile_pool(name="scr", bufs=2) as scr, \
             tc.tile_pool(name="sg", bufs=2, space="PSUM") as sg, \
             tc.tile_pool(name="pot", bufs=1, space="PSUM") as pot, \
             tc.tile_pool(name="pss", bufs=1, space="PSUM") as pss, \
             tc.tile_pool(name="ps", bufs=3, space="PSUM") as ps:

            # One ACT table set covers Exp/Ln/Square/Copy: load it up front.
            nc.scalar.add_instruction(mybir.InstLoadActFuncSet(
                name=nc.get_next_instruction_name(),
                ins=[], outs=[], act_func_set_id=nle_set))

            ones_col = cst.tile([128, 1], bf16, tag="ones_col")
            nc.gpsimd.memset(ones_col[:], 1.0)
            ones_row = cst.tile([1, 128], bf16, tag="ones_row")
            nc.gpsimd.memset(ones_row[:], 1.0)
            keeps_t = cst.tile([128, max(npat, 1), QT], bf16, tag="keeps")
            for i in range(npat):
                nc.sync.dma_start(keeps_t[:, i, :], keeps_d.ap()[i])
            lam_t = cst.tile([1, HLOC], f32, tag="lam")
            for h in range(HLOC):
                nc.sync.dma_start(lam_t[:, h:h + 1], lam_d.ap()[h])
            eps_t = cst.tile([1, 1], f32, tag="eps")
            nc.gpsimd.memset(eps_t[:], EPS)

            if repeat > 1:
                ctr = cst.tile([1, 1], f32, tag="ctr")
                nc.gpsimd.memset(ctr[:], 0.0)
            rep_ctx = tc.For_i(0, repeat, 1) if repeat > 1 else None
            if rep_ctx is not None:
                rep_ctx.__enter__()
                nc.vector.tensor_scalar_add(ctr[:], ctr[:], 1.0)

            def emit_body():
                # input DMAs issue from the Pool queue so they don't wait
                # behind the previous iteration's output DMAs on SP; wk/wq of
                # head 0 lead so the first KT matmuls start ASAP
                xT = xtp.tile([128, NEC, S], bf16, tag="xT")
                wtiles = []
                for h in range(HLOC):
                    wq_t = wts.tile([128, NEC, D], bf16, tag="wq", name=f"wq{h}")
                    wk_t = wts.tile([128, NEC, D], bf16, tag="wk", name=f"wk{h}")
                    wv_t = wts.tile([128, NEC, D], bf16, tag="wv", name=f"wv{h}")
                    wo_t = wts.tile([128, NDC, E], bf16, tag="wo", name=f"wo{h}")
                    wtiles.append((wq_t, wk_t, wv_t, wo_t))
                for ec in range(NEC):
                    nc.gpsimd.dma_start(wtiles[0][1][:, ec, :], wk_d.ap()[0, ec * 128:(ec + 1) * 128, :])
                for ec in range(NEC):
                    nc.gpsimd.dma_start(xT[:, ec, :], xt_d.ap()[ec * 128:(ec + 1) * 128, :])
                for h in range(HLOC):
                    wq_t, wk_t, wv_t, wo_t = wtiles[h]
                    for ec in range(NEC):
                        if h > 0:
                            nc.gpsimd.dma_start(wk_t[:, ec, :], wk_d.ap()[h, ec * 128:(ec + 1) * 128, :])
                        nc.gpsimd.dma_start(wq_t[:, ec, :], wq_d.ap()[h, ec * 128:(ec + 1) * 128, :])
                        nc.gpsimd.dma_start(wv_t[:, ec, :], wv_d.ap()[h, ec * 128:(ec + 1) * 128, :])
                    for dc in range(NDC):
                        nc.gpsimd.dma_start(wo_t[:, dc, :], wo_d.ap()[h, dc * 128:(dc + 1) * 128, :])

                # ---- software pipeline (per step t of the tile loop) ----
                #  B1(t-2): rms row math (ms matmuls + Ln/Exp)
                #  scores/exp/mask/sums(t); rec/g rows (DVE)
                #  AV(t-1) with the g-broadcast matmul for t injected after
                #    its first chain; combine(t) on Pool+DVE; sq/OT drains
                #  qt(t+1); B2(t-2): rr broadcast + outproj + store
                pend = {"A1": None, "A2": None, "B": None}

                def emit_tailA_AV(st, pre=None, inject=None):
                    h_, q0_, kcs_, qlo_, Wt_, V_, wo_t_ = st
                    nk_ = len(kcs_)
                    ot_ps = pot.tile([128, NDC, QT], f32, tag="ot")
                    for dc in range(NDC):
                        for i, kc in enumerate(kcs_):
                            lo = qlo_[kc]
                            nc.tensor.matmul(
                                ot_ps[:, dc, lo:QT],
                                V_[:, kc, dc * 128:(dc + 1) * 128],
                                Wt_[:, kc, lo:QT],
                                start=(i == 0), stop=(i == nk_ - 1))
                        if dc == 0 and pre is not None:
                            pre()
                            pre = None
                        if dc == 2 and inject is not None:
                            inject()
                            inject = None
                    if pre is not None:
                        pre()
                    if inject is not None:
                        inject()
                    return (h_, q0_, ot_ps, wo_t_)

                def emit_tailA_drains(ctx):
                    h_, q0_, ot_ps, wo_t_ = ctx
                    sq_sb = scr.tile([128, NDC, QT], bf16, tag="sq")
                    nc.scalar.activation(sq_sb[:], ot_ps[:], AF.Square,
                                         scale=float(1.0 / np.sqrt(D)))
                    ot_sb = scr.tile([128, NDC, QT], bf16, tag="otsb")
                    nc.vector.tensor_copy(ot_sb[:], ot_ps[:])
                    return (h_, q0_, sq_sb, ot_sb, wo_t_)

                def emit_tailB1(st):
                    h_, q0_, sq_sb_, ot_sb_, wo_t_ = st
                    ms_ps = pss.tile([1, QT], f32, tag="sums", name="ms_ps")
                    for dc in range(NDC):
                        nc.tensor.matmul(ms_ps[:], ones_col[:], sq_sb_[:, dc, :],
                                         start=(dc == 0), stop=(dc == NDC - 1))
                    lnm = scr.tile([1, QT], f32, tag="lnm")
                    nc.scalar.activation(lnm[:], ms_ps[:], AF.Ln, bias=eps_t[:])
                    rr = scr.tile([1, QT], bf16, tag="rr")
                    nc.scalar.activation(rr[:], lnm[:], AF.Exp, scale=-0.5)
                    return rr

                def emit_tailB2(st, rr):
                    h_, q0_, sq_sb_, ot_sb_, wo_t_ = st
                    rr_ps = ps.tile([128, QT], f32, tag="mmps", name="rrps")
                    nc.tensor.matmul(rr_ps[:], ones_row[:], rr[:], start=True, stop=True)
                    rr_bc = scr.tile([128, 1, QT], bf16, tag="rrbc")
                    nc.vector.tensor_copy(rr_bc[:, 0, :], rr_ps[:])
                    for ep in range(NEC // 2):
                        ops = ps.tile([128, 2, QT], f32, tag="mmps", name="outps")
                        for j in range(2):
                            ec = ep * 2 + j
                            for dc in range(NDC):
                                nc.tensor.matmul(
                                    ops[:, j, :], wo_t_[:, dc, ec * 128:(ec + 1) * 128],
                                    ot_sb_[:, dc, :],
                                    start=(dc == 0), stop=(dc == NDC - 1))
                        out_sb = scr.tile([128, 2, QT], f32, tag="outsb")
                        nc.vector.tensor_tensor(
                            out=out_sb[:], in0=ops[:],
                            in1=rr_bc[:, 0:1, :].to_broadcast([128, 2, QT]),
                            op=ALU.mult)
                        nc.sync.dma_start(
                            out_d.ap()[h_, ep * 256:(ep + 1) * 256, q0_:q0_ + QT]
                                .rearrange("(j p) q -> p j q", p=128),
                            out_sb[:])

                for h in range(HLOC):
                    wq_t, wk_t, wv_t, wo_t = wtiles[h]

                    # KT[d, k] = Wk.T @ xT   (bf16, drained on DVE)
                    KT = ktp.tile([128, NDC, S], bf16, tag="KT")
                    for dc in range(NDC):
                        for kt in range(4):
                            kps = ps.tile([128, 512], f32, tag="mmps", name="kps")
                            for ec in range(NEC):
                                nc.tensor.matmul(
                                    kps[:],
                                    wk_t[:, ec, dc * 128:(dc + 1) * 128],
                                    xT[:, ec, kt * 512:(kt + 1) * 512],
                                    start=(ec == 0), stop=(ec == NEC - 1))
                            nc.vector.tensor_copy(KT[:, dc, kt * 512:(kt + 1) * 512], kps[:])

                    # QT[d, q] = Wq.T @ xT for the whole head (bf16)
                    qt_t = qtp.tile([128, NDC, S], bf16, tag="qt")
                    for dc in range(NDC):
                        for kt in range(4):
                            qps = ps.tile([128, 512], f32, tag="mmps", name="qps")
                            for ec in range(NEC):
                                nc.tensor.matmul(
                                    qps[:],
                                    wq_t[:, ec, dc * 128:(dc + 1) * 128],
                                    xT[:, ec, kt * 512:(kt + 1) * 512],
                                    start=(ec == 0), stop=(ec == NEC - 1))
                            nc.scalar.activation(qt_t[:, dc, kt * 512:(kt + 1) * 512],
                                                 qps[:], AF.Copy)

                    # V[k, d] = x @ Wv   (bf16, drained on ACT)
                    V = vtp.tile([128, NKC, D], bf16, tag="V")
                    for sc in range(NSC):
                        vps = ps.tile([128, D], f32, tag="mmps", name="vps")
                        for ec in range(NEC):
                            nc.tensor.matmul(
                                vps[:],
                                xT[:, ec, sc * 128:(sc + 1) * 128],
                                wv_t[:, ec, :],
                                start=(ec == 0), stop=(ec == NEC - 1))
                        nc.vector.tensor_copy(V[:, sc, :], vps[:])

                    for t in range(NQT):
                        q0 = t * QT
                        diag_kcs = [kc for kc in range(NKC) if status[t][kc] >= 0]
                        full_kcs = [kc for kc in range(NKC) if status[t][kc] == FULL]
                        kcs = diag_kcs + full_kcs
                        nk = len(kcs)
                        # half-visible diagonal chunks compute only q[QT/2:)
                        # (needs a trailing full-width chunk to close groups)
                        qlo = {kc: (QT // 2 if (status[t][kc] >= 0
                                                and hv_pat[status[t][kc]]
                                                and full_kcs) else 0)
                               for kc in kcs}

                        # rms row math for tile t-2 (sq ready; before exps(t)
                        # enter the ACT queue so Ln/Exp aren't delayed)
                        rrB = None
                        if pend["B"] is not None:
                            rrB = emit_tailB1(pend["B"])

                        Et = epool.tile([128, NKC, 2, QT], bf16, tag="E")
                        Wt = wpool.tile([128, NKC, QT], bf16, tag="W")
                        sums_ps = pss.tile([1, 2, QT], f32, tag="sums")

                        # scores -> exp -> (mask); sums lag two chunks so PE
                        # never waits on the ACT exp latency
                        def emit_sums(i, kc):
                            lo = qlo[kc]
                            if lo == 0:
                                nc.tensor.matmul(
                                    sums_ps[:, :, :], ones_col[:], Et[:, kc, :, :],
                                    start=(i == 0), stop=(i == nk - 1))
                            else:
                                for half in (0, 1):
                                    nc.tensor.matmul(
                                        sums_ps[:, half, lo:QT], ones_col[:],
                                        Et[:, kc, half, lo:QT],
                                        start=(i == 0), stop=(i == nk - 1))

                        for i, kc in enumerate(kcs):
                            lo = qlo[kc]
                            sp = sg.tile([128, 2, QT], f32, tag="sp")
                            for half in (0, 1):
                                for j in (0, 1):
                                    dc = half * 2 + j
                                    nc.tensor.matmul(
                                        sp[:, half, lo:QT],
                                        KT[:, dc, kc * 128:(kc + 1) * 128],
                                        qt_t[:, dc, q0 + lo:q0 + QT],
                                        start=(j == 0), stop=(j == 1))
                            nc.scalar.activation(Et[:, kc, :, lo:QT],
                                                 sp[:, :, lo:QT], AF.Exp, scale=SCALE)
                            pat = status[t][kc]
                            if pat >= 0:
                                nc.vector.tensor_tensor(
                                    out=Et[:, kc, :, lo:QT], in0=Et[:, kc, :, lo:QT],
                                    in1=keeps_t[:, pat:pat + 1, lo:QT].to_broadcast(
                                        [128, 2, QT - lo]),
                                    op=ALU.mult)
                            if i >= 3:
                                emit_sums(i - 3, kcs[i - 3])

                        g_row = scr.tile([1, QT], bf16, tag="g")
                        g_bc = scr.tile([128, 1, QT], bf16, tag="gbc")

                        # sums tail + per-q combine scalar g = lam * s1 / s2,
                        # deferred into the AV stream so AV work covers the
                        # trailing exp latencies
                        def emit_sums_tail(emit_sums_=emit_sums, kcs_=kcs,
                                           nk_=nk, sums_ps_=sums_ps,
                                           g_row_=g_row, h_=h):
                            for i in range(max(0, nk_ - 3), nk_):
                                emit_sums_(i, kcs_[i])
                            rec = scr.tile([1, QT], f32, tag="rec")
                            nc.vector.reciprocal(rec[:], sums_ps_[:, 1, :])
                            nc.vector.scalar_tensor_tensor(
                                out=g_row_[:], in0=sums_ps_[:, 0, :],
                                scalar=lam_t[:, h_:h_ + 1], in1=rec[:],
                                op0=ALU.mult, op1=ALU.mult)

                        def emit_g_bc(g_row_=g_row, g_bc_=g_bc):
                            g_ps = ps.tile([128, QT], f32, tag="mmps", name="gps")
                            nc.tensor.matmul(g_ps[:], ones_row[:], g_row_[:],
                                             start=True, stop=True)
                            nc.vector.tensor_copy(g_bc_[:, 0, :], g_ps[:])

                        ctxA = None
                        if pend["A2"] is not None:
                            ctxA = emit_tailA_AV(pend["A2"], pre=emit_sums_tail,
                                                 inject=emit_g_bc)
                        else:
                            emit_sums_tail()
                            emit_g_bc()
                        ctxB = emit_tailA_drains(ctxA) if ctxA is not None else None

                        # W = E1 - G.*E2 over kc pairs (multiplies
                        # alternate Pool/DVE; contiguous kcs share one op)
                        pairs = []
                        ii = 0
                        while ii < nk:
                            if (ii + 1 < nk and kcs[ii + 1] == kcs[ii] + 1
                                    and qlo[kcs[ii]] == 0 and qlo[kcs[ii + 1]] == 0):
                                pairs.append((kcs[ii], 2, 0))
                                ii += 2
                            else:
                                pairs.append((kcs[ii], 1, qlo[kcs[ii]]))
                                ii += 1
                        for i, (kc, w_, lo) in enumerate(pairs):
                            tmp = scr.tile([128, 2, QT], bf16, tag="tmp")
                            eng = nc.vector if i % 2 == 0 else nc.gpsimd
                            eng.tensor_tensor(
                                out=tmp[:, 0:w_, lo:QT],
                                in0=g_bc[:, 0:1, lo:QT].to_broadcast(
                                    [128, w_, QT - lo]),
                                in1=Et[:, kc:kc + w_, 1, lo:QT], op=ALU.mult)
                            nc.vector.tensor_tensor(
                                out=Wt[:, kc:kc + w_, lo:QT],
                                in0=Et[:, kc:kc + w_, 0, lo:QT],
                                in1=tmp[:, 0:w_, lo:QT], op=ALU.subtract)
                        if pend["B"] is not None:
                            emit_tailB2(pend["B"], rrB)
                        pend["B"] = ctxB
                        pend["A2"] = pend["A1"]
                        pend["A1"] = (h, q0, kcs, qlo, Wt, V, wo_t)

                # flush: two AV stages remain (last head's tiles 6 and 7)
                for _ in range(2):
                    rrB = emit_tailB1(pend["B"]) if pend["B"] is not None else None
                    ctxA = emit_tailA_AV(pend["A2"])
                    ctxB = emit_tailA_drains(ctxA)
                    if pend["B"] is not None:
                        emit_tailB2(pend["B"], rrB)
                    pend["B"] = ctxB
                    pend["A2"] = pend["A1"]
                    pend["A1"] = None
                rrB = emit_tailB1(pend["B"])
                emit_tailB2(pend["B"], rrB)
                pend["A2"] = pend["B"] = None

            for _u in range(unroll):
                emit_body()

            if rep_ctx is not None:
                rep_ctx.__exit__(None, None, None)
                nc.sync.dma_start(iters_d.ap()[:], ctr[:])

    nc.compile()
    return nc


_CACHE = {}


def _get_program(mask, repeat=1, unroll=1):
    key = (mask.tobytes(), repeat, unroll)
    if key not in _CACHE:
        status, pats = _analyze_mask(mask)
        hv_pat = [bool(p[:, :QT // 2].max() == 0) for p in pats]
        nc = _build(status, len(pats), hv_pat=hv_pat, repeat=repeat, unroll=unroll)
        _CACHE[key] = (nc, pats)
    return _CACHE[key]


def make_in_maps(x, mask, Wq, bq, Wk, bk, Wv, bv, lq1, lk1, lq2, lk2,
                 lam_init_p, rms_w, Wo, bo, repeat=1, unroll=1):
    x = np.asarray(x, np.float32)
    mask = np.asarray(mask, bool)
    Wq = np.asarray(Wq, np.float32)
    Wk = np.asarray(Wk, np.float32)
    Wv = np.asarray(Wv, np.float32)
    Wo = np.asarray(Wo, np.float32)
    for b_ in (bq, bk, bv):
        assert np.abs(np.asarray(b_)).max() == 0.0, "nonzero qkv bias unsupported"
    lam = (np.exp((np.asarray(lq1, np.float32) * np.asarray(lk1, np.float32)).sum(-1))
           - np.exp((np.asarray(lq2, np.float32) * np.asarray(lk2, np.float32)).sum(-1))
           + np.asarray(lam_init_p, np.float32))  # [H]
    woF = Wo.reshape(H, D, E) * ((1.0 - LAMBDA_INIT) * np.asarray(rms_w, np.float32))[:, :, None]

    nc, pats = _get_program(mask, repeat=repeat, unroll=unroll)
    if pats:
        keeps = np.stack(pats).astype(ml_dtypes.bfloat16)
    else:
        keeps = np.zeros((1, 128, QT), ml_dtypes.bfloat16)

    in_maps = []
    for c in range(NCORES):
        b = c // 4
        h0 = HLOC * (c % 4)
        in_maps.append({
            "xt": np.ascontiguousarray(x[b].T).astype(ml_dtypes.bfloat16),
            "wq": Wq[h0:h0 + HLOC].astype(ml_dtypes.bfloat16),
            "wk": Wk[h0:h0 + HLOC].astype(ml_dtypes.bfloat16),
            "wv": Wv[h0:h0 + HLOC].astype(ml_dtypes.bfloat16),
            "wo": woF[h0:h0 + HLOC].astype(ml_dtypes.bfloat16),
            "lam": np.ascontiguousarray(lam[h0:h0 + HLOC].astype(np.float32)[:, None, None]),
            "keeps": keeps,
        })
    return nc, in_maps


def gather(results, bo):
    out = np.zeros((B, S, E), np.float32)
    for c in range(NCORES):
        out[c // 4] += results[c]["out"].sum(axis=0).T
    out += np.asarray(bo, np.float32)[None, None, :]
    return out


def kernel(**inputs):
    nc, in_maps = make_in_maps(**inputs)
    res = run_bass_kernel_spmd(nc, in_maps, core_ids=list(range(NCORES)))
    return gather(res.results, inputs["bo"])


# revision 33
# speedup vs baseline: 1.1189x; 1.0079x over previous
"""Differential multi-head self-attention on 8 Trainium2 NeuronCores.

Sharding: core c handles batch b = c // 4 and heads {2*(c%4), 2*(c%4)+1}
(data parallel over batch, tensor parallel over heads). One SPMD Bass
program runs on all 8 cores; per-core differences flow through input data.

v2 design (all-bf16 matmuls, softmax halves combined before AV):
  host supplies xT = x[b].T in bf16 (pure layout prep - no transposes on PE)
  KT = Wk.T @ xT, QT per tile, V = x @ Wv     (bf16 matmuls, FWL weight loads)
  per q tile of 256 q rows, k-chunk-major:
    scores[k, 2*256q] for both halves -> one exp per chunk (ACT)
    sums[1, 512] += ones.T @ E        (rank-1-stationary matmul per chunk)
  g[q] = lam * s1[q] / s2[q]; G = broadcast(g) via rank-1 matmul
  W = E1 - G.*E2  (Pool multiply + DVE subtract, bf16)
    -- exploits RMS-norm scale invariance: O = W@V = s1 * (a1 - lam*a2)@V,
       and the per-row factor s1 cancels in O/rms(O).
  OT[d, q] += V_chunk.T @ W            (transpose-free AV orientation)
  rms: sq = (OT/sqrt(D))^2 (ACT), ms = ones.T @ sq (matmul), rr = rsqrt
  out.T[e, q] = (Wo.T @ OT) * rr       (per-q rr applied on the PSUM drain)
  DRAM out is [H, E, S]; the host transposes when gathering (free).
"""

import numpy as np
import ml_dtypes

import concourse.bass as bass
import concourse.mybir as mybir
import concourse.tile as tile
from concourse import bacc
from concourse.bass_utils import run_bass_kernel_spmd
from concourse.hw_specs import get_activation_tables

B, S, E, H, D = 2, 2048, 512, 8, 512
HALF = D // 2
HLOC = 2            # heads per core
NCORES = 8
QT = 256            # q tile
NQT = S // QT       # 8
KC = 128            # k chunk
NKC = S // KC       # 16
NDC = D // 128      # 4
NEC = E // 128      # 4
NSC = S // 128      # 16
SCALE = 1.0 / float(np.sqrt(HALF))
EPS = float(np.finfo(np.float32).eps)
LAMBDA_INIT = 0.8

f32 = mybir.dt.float32
bf16 = mybir.dt.bfloat16
AF = mybir.ActivationFunctionType
ALU = mybir.AluOpType

SKIP, FULL = -1, -2


def _analyze_mask(mask):
    """Per (q-tile, k-chunk) block status: SKIP / FULL / keep-pattern index."""
    status = [[SKIP] * NKC for _ in range(NQT)]
    pats = []
    pat_idx = {}
    for t in range(NQT):
        for kc in range(NKC):
            blk = mask[t * QT:(t + 1) * QT, kc * KC:(kc + 1) * KC]  # [256 q, 128 k]
            if blk.all():
                status[t][kc] = SKIP
            elif not blk.any():
                status[t][kc] = FULL
            else:
                keep = (~blk).T.astype(np.float32)  # [128 k, 256 q]
                key = keep.tobytes()
                if key not in pat_idx:
                    pat_idx[key] = len(pats)
                    pats.append(keep)
                status[t][kc] = pat_idx[key]
    return status, pats


def _build(status, npat, hv_pat=(), repeat=1, unroll=1):  # noqa: C901
    nc = bacc.Bacc("TRN2", target_bir_lowering=False, debug=False)

    xt_d = nc.dram_tensor("xt", [E, S], bf16, kind="ExternalInput")
    wq_d = nc.dram_tensor("wq", [HLOC, E, D], bf16, kind="ExternalInput")
    wk_d = nc.dram_tensor("wk", [HLOC, E, D], bf16, kind="ExternalInput")
    wv_d = nc.dram_tensor("wv", [HLOC, E, D], bf16, kind="ExternalInput")
    wo_d = nc.dram_tensor("wo", [HLOC, D, E], bf16, kind="ExternalInput")
    lam_d = nc.dram_tensor("lam", [HLOC, 1, 1], f32, kind="ExternalInput")
    keeps_d = nc.dram_tensor("keeps", [npat, 128, QT], bf16, kind="ExternalInput")
    out_d = nc.dram_tensor("out", [HLOC, E, S], f32, kind="ExternalOutput")
    iters_d = nc.dram_tensor("iters", [1, 1], f32, kind="ExternalOutput") if repeat > 1 else None

    act_sets = list(get_activation_tables(nc.m.arch).keys())
    nle_set = act_sets.index("natural_log_exp_and_others")

    with tile.TileContext(nc) as tc:
        with tc.tile_pool(name="cst", bufs=1) as cst, \
             tc.tile_pool(name="xtp", bufs=1) as xtp, \
             tc.tile_pool(name="ktp", bufs=2) as ktp, \
             tc.tile_pool(name="vtp", bufs=2) as vtp, \
             tc.tile_pool(name="wts", bufs=2) as wts, \
             tc.tile_pool(name="qtp", bufs=1) as qtp, \
             tc.tile_pool(name="epool", bufs=2) as epool, \
             tc.tile_pool(name="wpool", bufs=3) as wpool, \
             tc.tile_pool(name="scr", bufs=2) as scr, \
             tc.tile_pool(name="sg", bufs=2, space="PSUM") as sg, \
             tc.tile_pool(name="pot", bufs=1, space="PSUM") as pot, \
             tc.tile_pool(name="pss", bufs=1, space="PSUM") as pss, \
             tc.tile_pool(name="ps", bufs=3, space="PSUM") as ps:

            # One ACT table set covers Exp/Ln/Square/Copy: load it up front.
            nc.scalar.add_instruction(mybir.InstLoadActFuncSet(
                name=nc.get_next_instruction_name(),
                ins=[], outs=[], act_func_set_id=nle_set))

            ones_col = cst.tile([128, 1], bf16, tag="ones_col")
            nc.gpsimd.memset(ones_col[:], 1.0)
            ones_row = cst.tile([1, 128], bf16, tag="ones_row")
            nc.gpsimd.memset(ones_row[:], 1.0)
            keeps_t = cst.tile([128, max(npat, 1), QT], bf16, tag="keeps")
            for i in range(npat):
                nc.sync.dma_start(keeps_t[:, i, :], keeps_d.ap()[i])
            lam_t = cst.tile([1, HLOC], f32, tag="lam")
            for h in range(HLOC):
                nc.sync.dma_start(lam_t[:, h:h + 1], lam_d.ap()[h])
            eps_t = cst.tile([1, 1], f32, tag="eps")
            nc.gpsimd.memset(eps_t[:], EPS)

            if repeat > 1:
                ctr = cst.tile([1, 1], f32, tag="ctr")
                nc.gpsimd.memset(ctr[:], 0.0)
            rep_ctx = tc.For_i(0, repeat, 1) if repeat > 1 else None
            if rep_ctx is not None:
                rep_ctx.__enter__()
                nc.vector.tensor_scalar_add(ctr[:], ctr[:], 1.0)

            def emit_body():
                # input DMAs issue from the Pool queue so they don't wait
                # behind the previous iteration's output DMAs on SP; wk/wq of
                # head 0 lead so the first KT matmuls start ASAP
                xT = xtp.tile([128, NEC, S], bf16, tag="xT")
                wtiles = []
                for h in range(HLOC):
                    wq_t = wts.tile([128, NEC, D], bf16, tag="wq", name=f"wq{h}")
                    wk_t = wts.tile([128, NEC, D], bf16, tag="wk", name=f"wk{h}")
                    wv_t = wts.tile([128, NEC, D], bf16, tag="wv", name=f"wv{h}")
                    wo_t = wts.tile([128, NDC, E], bf16, tag="wo", name=f"wo{h}")
                    wtiles.append((wq_t, wk_t, wv_t, wo_t))
                for ec in range(NEC):
                    nc.gpsimd.dma_start(wtiles[0][1][:, ec, :], wk_d.ap()[0, ec * 128:(ec + 1) * 128, :])
                for ec in range(NEC):
                    nc.gpsimd.dma_start(xT[:, ec, :], xt_d.ap()[ec * 128:(ec + 1) * 128, :])
                for h in range(HLOC):
                    wq_t, wk_t, wv_t, wo_t = wtiles[h]
                    for ec in range(NEC):
                        if h > 0:
                            nc.gpsimd.dma_start(wk_t[:, ec, :], wk_d.ap()[h, ec * 128:(ec + 1) * 128, :])
                        nc.gpsimd.dma_start(wq_t[:, ec, :], wq_d.ap()[h, ec * 128:(ec + 1) * 128, :])
                        nc.gpsimd.dma_start(wv_t[:, ec, :], wv_d.ap()[h, ec * 128:(ec + 1) * 128, :])
                    for dc in range(NDC):
                        nc.gpsimd.dma_start(wo_t[:, dc, :], wo_d.ap()[h, dc * 128:(dc + 1) * 128, :])

                # ---- software pipeline (per step t of the tile loop) ----
                #  B1(t-2): rms row math (ms matmuls + Ln/Exp)
                #  scores/exp/mask/sums(t); rec/g rows (DVE)
                #  AV(t-1) with the g-broadcast matmul for t injected after
                #    its first chain; combine(t) on Pool+DVE; sq/OT drains
                #  qt(t+1); B2(t-2): rr broadcast + outproj + store
                pend = {"A1": None, "A2": None, "B": None}

                def emit_tailA_AV(st, pre=None, inject=None):
                    h_, q0_, kcs_, qlo_, Wt_, V_, wo_t_ = st
                    nk_ = len(kcs_)
                    ot_ps = pot.tile([128, NDC, QT], f32, tag="ot")
                    for dc in range(NDC):
                        for i, kc in enumerate(kcs_):
                            lo = qlo_[kc]
                            nc.tensor.matmul(
                                ot_ps[:, dc, lo:QT],
                                V_[:, kc, dc * 128:(dc + 1) * 128],
                                Wt_[:, kc, lo:QT],
                                start=(i == 0), stop=(i == nk_ - 1))
                        if dc == 0 and pre is not None:
                            pre()
                            pre = None
                        if dc == 3 and inject is not None:
                            inject()
                            inject = None
                    if pre is not None:
                        pre()
                    if inject is not None:
                        inject()
                    return (h_, q0_, ot_ps, wo_t_)

                def emit_tailA_drains(ctx):
                    h_, q0_, ot_ps, wo_t_ = ctx
                    sq_sb = scr.tile([128, NDC, QT], bf16, tag="sq")
                    nc.scalar.activation(sq_sb[:], ot_ps[:], AF.Square,
                                         scale=float(1.0 / np.sqrt(D)))
                    ot_sb = scr.tile([128, NDC, QT], bf16, tag="otsb")
                    nc.vector.tensor_copy(ot_sb[:], ot_ps[:])
                    return (h_, q0_, sq_sb, ot_sb, wo_t_)

                def emit_tailB1(st):
                    h_, q0_, sq_sb_, ot_sb_, wo_t_ = st
                    ms_ps = pss.tile([1, QT], f32, tag="sums", name="ms_ps")
                    for dc in range(NDC):
                        nc.tensor.matmul(ms_ps[:], ones_col[:], sq_sb_[:, dc, :],
                                         start=(dc == 0), stop=(dc == NDC - 1))
                    lnm = scr.tile([1, QT], f32, tag="lnm")
                    nc.scalar.activation(lnm[:], ms_ps[:], AF.Ln, bias=eps_t[:])
                    rr = scr.tile([1, QT], bf16, tag="rr")
                    nc.scalar.activation(rr[:], lnm[:], AF.Exp, scale=-0.5)
                    return rr

                def emit_tailB2(st, rr):
                    h_, q0_, sq_sb_, ot_sb_, wo_t_ = st
                    rr_ps = ps.tile([128, QT], f32, tag="mmps", name="rrps")
                    nc.tensor.matmul(rr_ps[:], ones_row[:], rr[:], start=True, stop=True)
                    rr_bc = scr.tile([128, 1, QT], bf16, tag="rrbc")
                    nc.vector.tensor_copy(rr_bc[:, 0, :], rr_ps[:])
                    for ep in range(NEC // 2):
                        ops = ps.tile([128, 2, QT], f32, tag="mmps", name="outps")
                        for j in range(2):
                            ec = ep * 2 + j
                            for dc in range(NDC):
                                nc.tensor.matmul(
                                    ops[:, j, :], wo_t_[:, dc, ec * 128:(ec + 1) * 128],
                                    ot_sb_[:, dc, :],
                                    start=(dc == 0), stop=(dc == NDC - 1))
                        out_sb = scr.tile([128, 2, QT], f32, tag="outsb")
                        nc.vector.tensor_tensor(
                            out=out_sb[:], in0=ops[:],
                            in1=rr_bc[:, 0:1, :].to_broadcast([128, 2, QT]),
                            op=ALU.mult)
                        nc.sync.dma_start(
                            out_d.ap()[h_, ep * 256:(ep + 1) * 256, q0_:q0_ + QT]
                                .rearrange("(j p) q -> p j q", p=128),
                            out_sb[:])

                for h in range(HLOC):
                    wq_t, wk_t, wv_t, wo_t = wtiles[h]

                    # KT[d, k] = Wk.T @ xT   (bf16, drained on DVE)
                    KT = ktp.tile([128, NDC, S], bf16, tag="KT")
                    for dc in range(NDC):
                        for kt in range(4):
                            kps = ps.tile([128, 512], f32, tag="mmps", name="kps")
                            for ec in range(NEC):
                                nc.tensor.matmul(
                                    kps[:],
                                    wk_t[:, ec, dc * 128:(dc + 1) * 128],
                                    xT[:, ec, kt * 512:(kt + 1) * 512],
                                    start=(ec == 0), stop=(ec == NEC - 1))
                            nc.vector.tensor_copy(KT[:, dc, kt * 512:(kt + 1) * 512], kps[:])

                    # QT[d, q] = Wq.T @ xT for the whole head (bf16)
                    qt_t = qtp.tile([128, NDC, S], bf16, tag="qt")
                    for dc in range(NDC):
                        for kt in range(4):
                            qps = ps.tile([128, 512], f32, tag="mmps", name="qps")
                            for ec in range(NEC):
                                nc.tensor.matmul(
                                    qps[:],
                                    wq_t[:, ec, dc * 128:(dc + 1) * 128],
                                    xT[:, ec, kt * 512:(kt + 1) * 512],
                                    start=(ec == 0), stop=(ec == NEC - 1))
                            nc.scalar.activation(qt_t[:, dc, kt * 512:(kt + 1) * 512],
                                                 qps[:], AF.Copy)

                    # V[k, d] = x @ Wv   (bf16, drained on ACT)
                    V = vtp.tile([128, NKC, D], bf16, tag="V")
                    for sc in range(NSC):
                        vps = ps.tile([128, D], f32, tag="mmps", name="vps")
                        for ec in range(NEC):
                            nc.tensor.matmul(
                                vps[:],
                                xT[:, ec, sc * 128:(sc + 1) * 128],
                                wv_t[:, ec, :],
                                start=(ec == 0), stop=(ec == NEC - 1))
                        nc.vector.tensor_copy(V[:, sc, :], vps[:])

                    for t in range(NQT):
                        q0 = t * QT
                        diag_kcs = [kc for kc in range(NKC) if status[t][kc] >= 0]
                        full_kcs = [kc for kc in range(NKC) if status[t][kc] == FULL]
                        kcs = diag_kcs + full_kcs
                        nk = len(kcs)
                        # half-visible diagonal chunks compute only q[QT/2:)
                        # (needs a trailing full-width chunk to close groups)
                        qlo = {kc: (QT // 2 if (status[t][kc] >= 0
                                                and hv_pat[status[t][kc]]
                                                and full_kcs) else 0)
                               for kc in kcs}

                        # rms row math for tile t-2 (sq ready; before exps(t)
                        # enter the ACT queue so Ln/Exp aren't delayed)
                        rrB = None
                        if pend["B"] is not None:
                            rrB = emit_tailB1(pend["B"])

                        Et = epool.tile([128, NKC, 2, QT], bf16, tag="E")
                        Wt = wpool.tile([128, NKC, QT], bf16, tag="W")
                        sums_ps = pss.tile([1, 2, QT], f32, tag="sums")

                        # scores -> exp -> (mask); sums lag two chunks so PE
                        # never waits on the ACT exp latency
                        def emit_sums(i, kc):
                            lo = qlo[kc]
                            if lo == 0:
                                nc.tensor.matmul(
                                    sums_ps[:, :, :], ones_col[:], Et[:, kc, :, :],
                                    start=(i == 0), stop=(i == nk - 1))
                            else:
                                for half in (0, 1):
                                    nc.tensor.matmul(
                                        sums_ps[:, half, lo:QT], ones_col[:],
                                        Et[:, kc, half, lo:QT],
                                        start=(i == 0), stop=(i == nk - 1))

                        for i, kc in enumerate(kcs):
                            lo = qlo[kc]
                            sp = sg.tile([128, 2, QT], f32, tag="sp")
                            for half in (0, 1):
                                for j in (0, 1):
                                    dc = half * 2 + j
                                    nc.tensor.matmul(
                                        sp[:, half, lo:QT],
                                        KT[:, dc, kc * 128:(kc + 1) * 128],
                                        qt_t[:, dc, q0 + lo:q0 + QT],
                                        start=(j == 0), stop=(j == 1))
                            nc.scalar.activation(Et[:, kc, :, lo:QT],
                                                 sp[:, :, lo:QT], AF.Exp, scale=SCALE)
                            pat = status[t][kc]
                            if pat >= 0:
                                nc.vector.tensor_tensor(
                                    out=Et[:, kc, :, lo:QT], in0=Et[:, kc, :, lo:QT],
                                    in1=keeps_t[:, pat:pat + 1, lo:QT].to_broadcast(
                                        [128, 2, QT - lo]),
                                    op=ALU.mult)
                            if i >= 3:
                                emit_sums(i - 3, kcs[i - 3])

                        g_row = scr.tile([1, QT], bf16, tag="g")
                        g_bc = scr.tile([128, 1, QT], bf16, tag="gbc")

                        # sums tail + per-q combine scalar g = lam * s1 / s2,
                        # deferred into the AV stream so AV work covers the
                        # trailing exp latencies
                        def emit_sums_tail(emit_sums_=emit_sums, kcs_=kcs,
                                           nk_=nk, sums_ps_=sums_ps,
                                           g_row_=g_row, h_=h):
                            for i in range(max(0, nk_ - 3), nk_):
                                emit_sums_(i, kcs_[i])
                            rec = scr.tile([1, QT], f32, tag="rec")
                            nc.vector.reciprocal(rec[:], sums_ps_[:, 1, :])
                            nc.vector.scalar_tensor_tensor(
                                out=g_row_[:], in0=sums_ps_[:, 0, :],
                                scalar=lam_t[:, h_:h_ + 1], in1=rec[:],
                                op0=ALU.mult, op1=ALU.mult)

                        def emit_g_bc(g_row_=g_row, g_bc_=g_bc):
                            g_ps = ps.tile([128, QT], f32, tag="mmps", name="gps")
                            nc.tensor.matmul(g_ps[:], ones_row[:], g_row_[:],
                                             start=True, stop=True)
                            nc.vector.tensor_copy(g_bc_[:, 0, :], g_ps[:])

                        ctxA = None
                        if pend["A2"] is not None:
                            ctxA = emit_tailA_AV(pend["A2"], pre=emit_sums_tail,
                                                 inject=emit_g_bc)
                        else:
                            emit_sums_tail()
                            emit_g_bc()
                        ctxB = emit_tailA_drains(ctxA) if ctxA is not None else None

                        # W = E1 - G.*E2 over kc pairs (multiplies
                        # alternate Pool/DVE; contiguous kcs share one op)
                        pairs = []
                        ii = 0
                        while ii < nk:
                            if (ii + 1 < nk and kcs[ii + 1] == kcs[ii] + 1
                                    and qlo[kcs[ii]] == 0 and qlo[kcs[ii + 1]] == 0):
                                pairs.append((kcs[ii], 2, 0))
                                ii += 2
                            else:
                                pairs.append((kcs[ii], 1, qlo[kcs[ii]]))
                                ii += 1
                        for i, (kc, w_, lo) in enumerate(pairs):
                            tmp = scr.tile([128, 2, QT], bf16, tag="tmp")
                            eng = nc.vector if i % 2 == 0 else nc.gpsimd
                            eng.tensor_tensor(
                                out=tmp[:, 0:w_, lo:QT],
                                in0=g_bc[:, 0:1, lo:QT].to_broadcast(
                                    [128, w_, QT - lo]),
                                in1=Et[:, kc:kc + w_, 1, lo:QT], op=ALU.mult)
                            nc.vector.tensor_tensor(
                                out=Wt[:, kc:kc + w_, lo:QT],
                                in0=Et[:, kc:kc + w_, 0, lo:QT],
                                in1=tmp[:, 0:w_, lo:QT], op=ALU.subtract)
                        if pend["B"] is not None:
                            emit_tailB2(pend["B"], rrB)
                        pend["B"] = ctxB
                        pend["A2"] = pend["A1"]
                        pend["A1"] = (h, q0, kcs, qlo, Wt, V, wo_t)

                # flush: two AV stages remain (last head's tiles 6 and 7)
                for _ in range(2):
                    rrB = emit_tailB1(pend["B"]) if pend["B"] is not None else None
                    ctxA = emit_tailA_AV(pend["A2"])
                    ctxB = emit_tailA_drains(ctxA)
                    if pend["B"] is not None:
                        emit_tailB2(pend["B"], rrB)
                    pend["B"] = ctxB
                    pend["A2"] = pend["A1"]
                    pend["A1"] = None
                rrB = emit_tailB1(pend["B"])
                emit_tailB2(pend["B"], rrB)
                pend["A2"] = pend["B"] = None

            for _u in range(unroll):
                emit_body()

            if rep_ctx is not None:
                rep_ctx.__exit__(None, None, None)
                nc.sync.dma_start(iters_d.ap()[:], ctr[:])

    nc.compile()
    return nc


_CACHE = {}


def _get_program(mask, repeat=1, unroll=1):
    key = (mask.tobytes(), repeat, unroll)
    if key not in _CACHE:
        status, pats = _analyze_mask(mask)
        hv_pat = [bool(p[:, :QT // 2].max() == 0) for p in pats]
        nc = _build(status, len(pats), hv_pat=hv_pat, repeat=repeat, unroll=unroll)
        _CACHE[key] = (nc, pats)
    return _CACHE[key]


def make_in_maps(x, mask, Wq, bq, Wk, bk, Wv, bv, lq1, lk1, lq2, lk2,
                 lam_init_p, rms_w, Wo, bo, repeat=1, unroll=1):
    x = np.asarray(x, np.float32)
    mask = np.asarray(mask, bool)
    Wq = np.asarray(Wq, np.float32)
    Wk = np.asarray(Wk, np.float32)
    Wv = np.asarray(Wv, np.float32)
    Wo = np.asarray(Wo, np.float32)
    for b_ in (bq, bk, bv):
        assert np.abs(np.asarray(b_)).max() == 0.0, "nonzero qkv bias unsupported"
    lam = (np.exp((np.asarray(lq1, np.float32) * np.asarray(lk1, np.float32)).sum(-1))
           - np.exp((np.asarray(lq2, np.float32) * np.asarray(lk2, np.float32)).sum(-1))
           + np.asarray(lam_init_p, np.float32))  # [H]
    woF = Wo.reshape(H, D, E) * ((1.0 - LAMBDA_INIT) * np.asarray(rms_w, np.float32))[:, :, None]

    nc, pats = _get_program(mask, repeat=repeat, unroll=unroll)
    if pats:
        keeps = np.stack(pats).astype(ml_dtypes.bfloat16)
    else:
        keeps = np.zeros((1, 128, QT), ml_dtypes.bfloat16)

    in_maps = []
    for c in range(NCORES):
        b = c // 4
        h0 = HLOC * (c % 4)
        in_maps.append({
            "xt": np.ascontiguousarray(x[b].T).astype(ml_dtypes.bfloat16),
            "wq": Wq[h0:h0 + HLOC].astype(ml_dtypes.bfloat16),
            "wk": Wk[h0:h0 + HLOC].astype(ml_dtypes.bfloat16),
            "wv": Wv[h0:h0 + HLOC].astype(ml_dtypes.bfloat16),
            "wo": woF[h0:h0 + HLOC].astype(ml_dtypes.bfloat16),
            "lam": np.ascontiguousarray(lam[h0:h0 + HLOC].astype(np.float32)[:, None, None]),
            "keeps": keeps,
        })
    return nc, in_maps


def gather(results, bo):
    out = np.zeros((B, S, E), np.float32)
    for c in range(NCORES):
        out[c // 4] += results[c]["out"].sum(axis=0).T
    out += np.asarray(bo, np.float32)[None, None, :]
    return out


def kernel(**inputs):
    nc, in_maps = make_in_maps(**inputs)
    res = run_bass_kernel_spmd(nc, in_maps, core_ids=list(range(NCORES)))
    return gather(res.results, inputs["bo"])
